# revision 15
# baseline (speedup 1.0000x reference)
"""ChildSum TreeLSTM on a fixed 8-ary heap tree (N=65536), 8 TRN2 NeuronCores.

Tree facts (hardcoded, verified against the reference tree builder):
  parent(i) = (i-1)//8; node levels form contiguous ranges:
    L0 leaves [8192,65536), L1 [1024,8192), L2 [128,1024), L3 [16,128),
    L4 [2,16), L5 {1}, L6 {0}.  Children of node p are [8p+1, 8p+9).

Shard scheme (core k of 8) — chosen so every core's children columns are its
own previously computed columns (zero cross-core traffic except one tiny
AllGather of L2 results):
  S_leaf: 7168 cols -> nodes [8201+7168k, 8201+7168(k+1))  (>=65536 -> zero pads)
  S_L1:    896 cols -> nodes [1025+896k, 1921+896k)  (core 7 last col = node 8192,
           a leaf: with zeroed pad children the parent pipeline reduces to the
           leaf equations, so it computes the right value automatically)
  S_L2:    112 cols -> nodes [128+112k, 240+112k)
  S_tail:  137 cols -> nodes [0,128) + {1024} + [8193,8201)  (replicated)

On-device layout is feature-major node-order: h/c/x stored [128 feats, nodes].
Matmul operands are bf16 (fp32 matmul on TRN2 is ~4x slower); PSUM and the c
path stay fp32.  i/o/u gates exploit child-sum linearity twice: the 8-child
h-sum is ONE contiguous DVE reduce, then a single U matmul per gate
(strided-rhs matmuls stall the PE ~2-3x, and strided DVE writes are worse).
Per-edge forget gates use a broadcast (step-0) rhs for the parent x term.
"""
import numpy as np
import ml_dtypes

import concourse.bass as bass
import concourse.mybir as mybir
import concourse.tile as tile
from concourse import bacc
from concourse import bass_utils

F32 = mybir.dt.float32
BF16 = mybir.dt.bfloat16
NPBF = ml_dtypes.bfloat16
AF = mybir.ActivationFunctionType
H = 128
N = 65536
NCORE = 8
NLEAF = 7168
NL1 = 896
NL2 = 112
NTAIL = 137
NCOLS = NLEAF + NL1 + NL2 + NTAIL  # 8313
SB = 1024           # leaf superblock width
PB = 448            # parent block width
XI_L1 = 0           # xint column offsets
XI_L2 = 896
XI_TAIL = 1008      # nodes [0,128) at xint cols [1008,1136)
XI_1024 = 1136
XI_TLEAF = 1137
XI_W = 1145
# out column offsets
OC_LEAF = 0
OC_L1 = NLEAF
OC_L2 = NLEAF + NL1
OC_TAIL = NLEAF + NL1 + NL2          # nodes [0,128)
OC_1024 = OC_TAIL + 128
OC_TLEAF = OC_TAIL + 129


def _leaf_gates(nc, P, xa, xb, wc0, wc1, bias, width, outH, outC, mask=None):
    """Dense-only i/o/u gates -> h,c for `width` columns.  outH bf16, outC fp32."""
    def dense(g):
        p = P["psl"].tile([H, width], F32, tag="psl")
        for h0 in range(0, width, 512):
            w = min(512, width - h0)
            nc.tensor.matmul(p[:, h0:h0 + w], wc0[:, g * 128:(g + 1) * 128],
                             xa[:, h0:h0 + w], start=True, stop=False)
            nc.tensor.matmul(p[:, h0:h0 + w], wc1[:, g * 128:(g + 1) * 128],
                             xb[:, h0:h0 + w], start=False, stop=True)
        return p

    ps_i = dense(0)
    ps_u = dense(2)
    si = P["gt"].tile([H, width], BF16, tag="si")
    nc.scalar.activation(si, ps_i, AF.Sigmoid, bias=bias[:, 0:1])
    tu = P["gt"].tile([H, width], BF16, tag="tu")
    nc.scalar.activation(tu, ps_u, AF.Tanh, bias=bias[:, 2:3])
    if mask is not None:
        nc.vector.tensor_mul(si, si, mask)
    nc.vector.tensor_mul(outC, si, tu)
    ps_o = dense(1)
    so = P["gt"].tile([H, width], BF16, tag="so")
    nc.scalar.activation(so, ps_o, AF.Sigmoid, bias=bias[:, 1:2])
    tcx = P["gt"].tile([H, width], BF16, tag="tc")
    nc.scalar.activation(tcx, outC, AF.Tanh)
    nc.vector.tensor_mul(outH, so, tcx)


def _level(nc, P, xint0, xint1, wc0, wc1, u_iou, u_f, bias,
           xoff, npar, chH, chC, choff, outH, outC, oh):
    """One recurrence level, node-order children: children of local parent j at
    chH/chC cols [choff+8j, choff+8j+8).  chH/outH bf16; chC/outC fp32.
    i/o/u: h-sum of 8 children via one DVE reduce, then one U matmul per gate."""
    for pb0 in range(0, npar, PB):
        pw = min(PB, npar - pb0)
        ch_lo = choff + 8 * pb0
        # child h-sum (contiguous-inner reduce) -> bf16 for the U matmuls
        hsum = P["pt"].tile([H, PB], F32, tag="hsum")
        nc.vector.tensor_reduce(hsum[:, 0:pw],
                                chH[:, ch_lo:ch_lo + 8 * pw].rearrange("p (n e) -> p n e", e=8),
                                axis=mybir.AxisListType.X, op=mybir.AluOpType.add)
        hsb = P["pt"].tile([H, PB], BF16, tag="hsb")
        nc.vector.tensor_copy(hsb[:, 0:pw], hsum[:, 0:pw])
        sg = {}
        for g, nm in ((0, "i"), (2, "u"), (1, "o")):
            p = P["psa"].tile([H, pw], F32, tag="psa")
            nc.tensor.matmul(p, wc0[:, g * 128:(g + 1) * 128],
                             xint0[:, xoff + pb0:xoff + pb0 + pw], start=True, stop=False)
            nc.tensor.matmul(p, wc1[:, g * 128:(g + 1) * 128],
                             xint1[:, xoff + pb0:xoff + pb0 + pw], start=False, stop=False)
            nc.tensor.matmul(p, u_iou[:, g * 128:(g + 1) * 128], hsb[:, 0:pw],
                             start=False, stop=True)
            s = P["pt"].tile([H, pw], BF16, tag=f"s{nm}")
            nc.scalar.activation(s, p, AF.Tanh if g == 2 else AF.Sigmoid,
                                 bias=bias[:, g:g + 1])
            sg[nm] = s
        # per-child forget gates; fc grouped-sum
        fcs = P["pt"].tile([H, pw], F32, tag="fcs")
        for cb0 in range(0, 8 * pw, 512):
            cw = min(512, 8 * pw - cb0)
            npb = cw // 8
            pf = P["psf"].tile([H, cw], F32, tag="psf")
            xp0 = xint0[:, xoff + pb0 + cb0 // 8:xoff + pb0 + cb0 // 8 + npb]
            xp1 = xint1[:, xoff + pb0 + cb0 // 8:xoff + pb0 + cb0 // 8 + npb]
            nc.tensor.matmul(pf, wc0[:, 384:512],
                             xp0.unsqueeze(2).broadcast_to([H, npb, 8]), start=True, stop=False)
            nc.tensor.matmul(pf, wc1[:, 384:512],
                             xp1.unsqueeze(2).broadcast_to([H, npb, 8]), start=False, stop=False)
            nc.tensor.matmul(pf, u_f, chH[:, ch_lo + cb0:ch_lo + cb0 + cw],
                             start=False, stop=True)
            ft = P["fp"].tile([H, 512], BF16, tag="ft")
            nc.scalar.activation(ft[:, 0:cw], pf, AF.Sigmoid, bias=bias[:, 3:4])
            fct = P["fp"].tile([H, 512], BF16, tag="fct")
            nc.vector.tensor_mul(fct[:, 0:cw], ft[:, 0:cw],
                                 chC[:, ch_lo + cb0:ch_lo + cb0 + cw])
            nc.vector.tensor_reduce(fcs[:, cb0 // 8:cb0 // 8 + npb],
                                    fct[:, 0:cw].rearrange("p (n e) -> p n e", e=8),
                                    axis=mybir.AxisListType.X, op=mybir.AluOpType.add)
        # combine
        ct = P["pt"].tile([H, pw], BF16, tag="ct")
        nc.vector.tensor_mul(ct, sg["i"], sg["u"])
        cs = outC[:, oh + pb0:oh + pb0 + pw]
        nc.vector.tensor_add(cs, ct, fcs)
        tcx = P["pt"].tile([H, pw], BF16, tag="tcx")
        nc.scalar.activation(tcx, cs, AF.Tanh)
        hs = outH[:, oh + pb0:oh + pb0 + pw]
        nc.vector.tensor_mul(hs, sg["o"], tcx)


def build():
    nc = bacc.Bacc("TRN2", target_bir_lowering=False, debug=False, num_devices=NCORE)
    xT = nc.dram_tensor("xT", [256, NCOLS], BF16, kind="ExternalInput")
    wcat = nc.dram_tensor("wcat", [256, 512], BF16, kind="ExternalInput")
    uiou = nc.dram_tensor("uiou", [H, 384], BF16, kind="ExternalInput")
    uf = nc.dram_tensor("uf", [H, H], BF16, kind="ExternalInput")
    bias_d = nc.dram_tensor("bias", [H, 4], F32, kind="ExternalInput")
    mask_d = nc.dram_tensor("mask", [H, SB], BF16, kind="ExternalInput")
    h_out = nc.dram_tensor("h_out", [H, NCOLS], BF16, kind="ExternalOutput")
    c_out = nc.dram_tensor("c_out", [H, NCOLS], BF16, kind="ExternalOutput")

    with tile.TileContext(nc) as tc:
        with (
            tc.tile_pool(name="const", bufs=1) as const,
            tc.tile_pool(name="big", bufs=1) as big,
            tc.tile_pool(name="stream", bufs=3) as stream,
            tc.tile_pool(name="gt", bufs=3) as gt,
            tc.tile_pool(name="pt", bufs=3) as pt,
            tc.tile_pool(name="fp", bufs=3) as fp,
            tc.tile_pool(name="psl", bufs=2, space="PSUM") as psl,
            tc.tile_pool(name="psa", bufs=2, space="PSUM") as psa,
            tc.tile_pool(name="psf", bufs=2, space="PSUM") as psf,
            tc.tile_pool(name="dram", bufs=1, space="DRAM") as dram,
        ):
            P = {"psl": psl, "psa": psa, "psf": psf, "gt": gt, "pt": pt, "fp": fp}

            wc0 = const.tile([H, 512], BF16, tag="wc0")
            wc1 = const.tile([H, 512], BF16, tag="wc1")
            nc.sync.dma_start(wc0, wcat.ap()[0:128, :])
            nc.sync.dma_start(wc1, wcat.ap()[128:256, :])
            bias = const.tile([H, 4], F32, tag="bias")
            nc.sync.dma_start(bias, bias_d.ap())

            leafH = big.tile([H, NLEAF], BF16, tag="leafH")
            leafC = big.tile([H, NLEAF], BF16, tag="leafC")
            hL1 = big.tile([H, NL1], BF16, tag="hL1")
            cL1 = big.tile([H, NL1], BF16, tag="cL1")
            hL2 = big.tile([H, NL2], BF16, tag="hL2")
            cL2 = big.tile([H, NL2], BF16, tag="cL2")
            hS = big.tile([H, 1025], BF16, tag="hS")
            cS = big.tile([H, 1025], BF16, tag="cS")
            htl = big.tile([H, 8], BF16, tag="htl")
            ctl = big.tile([H, 8], BF16, tag="ctl")

            def leaf_sb(sb):
                xa = stream.tile([H, SB], BF16, tag="xa")
                xb = stream.tile([H, SB], BF16, tag="xb")
                nc.sync.dma_start(xa, xT.ap()[0:128, sb * SB:(sb + 1) * SB])
                nc.sync.dma_start(xb, xT.ap()[128:256, sb * SB:(sb + 1) * SB])
                _leaf_gates(nc, P, xa, xb, wc0, wc1, bias, SB,
                            leafH[:, sb * SB:(sb + 1) * SB], leafC[:, sb * SB:(sb + 1) * SB],
                            mask=mask if sb == NLEAF // SB - 1 else None)

            # ---- Half 0: leaves sb0-3 -> L1 pb0 -> L2 [0,56) -> AllGather#0 ----
            leaf_sb(0)
            # deferred const loads (not needed until L1 / last superblock)
            u_iou = const.tile([H, 384], BF16, tag="uiou")
            nc.sync.dma_start(u_iou, uiou.ap())
            u_f = const.tile([H, H], BF16, tag="uf")
            nc.sync.dma_start(u_f, uf.ap())
            mask = const.tile([H, SB], BF16, tag="mask")
            nc.sync.dma_start(mask, mask_d.ap())
            xint0 = const.tile([H, XI_W], BF16, tag="xint0")
            xint1 = const.tile([H, XI_W], BF16, tag="xint1")
            nc.sync.dma_start(xint0, xT.ap()[0:128, NLEAF:NCOLS])
            nc.sync.dma_start(xint1, xT.ap()[128:256, NLEAF:NCOLS])
            for sb in range(1, 4):
                leaf_sb(sb)

            _level(nc, P, xint0, xint1, wc0, wc1, u_iou, u_f, bias,
                   XI_L1, PB, leafH, leafC, 0, hL1, cL1, 0)
            _level(nc, P, xint0, xint1, wc0, wc1, u_iou, u_f, bias,
                   XI_L2, 56, hL1, cL1, 0, hL2, cL2, 0)
            agi0 = dram.tile([2, H, 56], BF16, tag="agi0")
            ago0 = dram.tile([NCORE, 2, H, 56], BF16, tag="ago0")
            nc.sync.dma_start(agi0[0], hL2[:, 0:56])
            nc.sync.dma_start(agi0[1], cL2[:, 0:56])
            nc.gpsimd.collective_compute(
                "AllGather", mybir.AluOpType.bypass,
                replica_groups=[list(range(NCORE))],
                ins=[agi0.opt()], outs=[ago0.opt()],
            )

            # ---- Half 1: leaves sb4-6 -> L1 pb1 -> L2 [56,112) -> AllGather#1 ----
            for sb in range(4, NLEAF // SB):
                leaf_sb(sb)
            _level(nc, P, xint0, xint1, wc0, wc1, u_iou, u_f, bias,
                   XI_L1 + PB, PB, leafH, leafC, 8 * PB, hL1, cL1, PB)
            _level(nc, P, xint0, xint1, wc0, wc1, u_iou, u_f, bias,
                   XI_L2 + 56, 56, hL1, cL1, 448, hL2, cL2, 56)
            agi1 = dram.tile([2, H, 56], BF16, tag="agi1")
            ago1 = dram.tile([NCORE, 2, H, 56], BF16, tag="ago1")
            nc.sync.dma_start(agi1[0], hL2[:, 56:112])
            nc.sync.dma_start(agi1[1], cL2[:, 56:112])
            nc.gpsimd.collective_compute(
                "AllGather", mybir.AluOpType.bypass,
                replica_groups=[list(range(NCORE))],
                ins=[agi1.opt()], outs=[ago1.opt()],
            )

            # ---- Tail leaves [8193,8201) + node 1024 (overlap the gathers) ----
            _leaf_gates(nc, P, xint0[:, XI_TLEAF:XI_TLEAF + 8], xint1[:, XI_TLEAF:XI_TLEAF + 8],
                        wc0, wc1, bias, 8, htl[:, 0:8], ctl[:, 0:8])
            nc.sync.dma_start(h_out.ap()[:, OC_TLEAF:OC_TLEAF + 8], htl)
            nc.sync.dma_start(c_out.ap()[:, OC_TLEAF:OC_TLEAF + 8], ctl)
            _level(nc, P, xint0, xint1, wc0, wc1, u_iou, u_f, bias,
                   XI_1024, 1, htl, ctl, 0, hS[:, 1024:1025], cS[:, 1024:1025], 0)
            nc.sync.dma_start(h_out.ap()[:, OC_1024:OC_1024 + 1], hS[:, 1024:1025])
            nc.sync.dma_start(c_out.ap()[:, OC_1024:OC_1024 + 1], cS[:, 1024:1025])

            # bulk output DMAs (fill the gather-wait window)
            nc.sync.dma_start(h_out.ap()[:, 0:NLEAF], leafH)
            nc.sync.dma_start(c_out.ap()[:, 0:NLEAF], leafC)
            nc.sync.dma_start(h_out.ap()[:, OC_L1:OC_L1 + NL1], hL1)
            nc.sync.dma_start(c_out.ap()[:, OC_L1:OC_L1 + NL1], cL1)
            nc.sync.dma_start(h_out.ap()[:, OC_L2:OC_L2 + NL2], hL2)
            nc.sync.dma_start(c_out.ap()[:, OC_L2:OC_L2 + NL2], cL2)

            # ---- land the gathers into the tail state ----
            hSv = hS[:, 128:1024].rearrange("p (b c) -> p b c", b=NCORE)
            cSv = cS[:, 128:1024].rearrange("p (b c) -> p b c", b=NCORE)
            nc.sync.dma_start(hSv[:, :, 0:56], ago0[:, 0].transpose([1, 0, 2]))
            nc.sync.dma_start(cSv[:, :, 0:56], ago0[:, 1].transpose([1, 0, 2]))
            nc.sync.dma_start(hSv[:, :, 56:112], ago1[:, 0].transpose([1, 0, 2]))
            nc.sync.dma_start(cSv[:, :, 56:112], ago1[:, 1].transpose([1, 0, 2]))

            # ---- Tail levels L3..L6 on gathered state ----
            for xo, np_, choff, olo in ((XI_TAIL + 16, 112, 129, 16),
                                        (XI_TAIL + 2, 14, 17, 2),
                                        (XI_TAIL + 1, 1, 9, 1),
                                        (XI_TAIL, 1, 1, 0)):
                _level(nc, P, xint0, xint1, wc0, wc1, u_iou, u_f, bias,
                       xo, np_, hS, cS, choff,
                       hS[:, olo:olo + np_], cS[:, olo:olo + np_], 0)
            nc.sync.dma_start(h_out.ap()[:, OC_TAIL:OC_TAIL + 128], hS[:, 0:128])
            nc.sync.dma_start(c_out.ap()[:, OC_TAIL:OC_TAIL + 128], cS[:, 0:128])
    nc.compile()
    return nc


_NC_CACHE = None


def _get_program():
    global _NC_CACHE
    if _NC_CACHE is None:
        _NC_CACHE = build()
    return _NC_CACHE


def _host_prep(x, W_iou, U_iou, b_iou, W_f, U_f, b_f):
    x = np.asarray(x, np.float32)
    xTg = np.ascontiguousarray(x.T.astype(NPBF))  # [256, 65536] bf16
    wcat = np.ascontiguousarray(
        np.concatenate([np.asarray(W_iou, np.float32).T,
                        np.asarray(W_f, np.float32).T], axis=1).astype(NPBF))
    uiou = np.ascontiguousarray(np.asarray(U_iou, np.float32).astype(NPBF))
    uf = np.ascontiguousarray(np.asarray(U_f, np.float32).astype(NPBF))
    b_iou = np.asarray(b_iou, np.float32)[0]
    b_f = np.asarray(b_f, np.float32)[0]
    bias = np.ascontiguousarray(
        np.stack([b_iou[0:128], b_iou[128:256], b_iou[256:384], b_f], axis=1))

    in_maps = []
    for k in range(NCORE):
        xk = np.empty((256, NCOLS), NPBF)
        lo = 8201 + NLEAF * k
        hi = min(lo + NLEAF, N)
        nreal = hi - lo
        xk[:, 0:nreal] = xTg[:, lo:hi]
        if nreal < NLEAF:
            xk[:, nreal:NLEAF] = 0.0
        xk[:, NLEAF:NLEAF + NL1] = xTg[:, 1025 + NL1 * k:1921 + NL1 * k]
        xk[:, OC_L2:OC_L2 + NL2] = xTg[:, 128 + NL2 * k:240 + NL2 * k]
        xk[:, OC_TAIL:OC_TAIL + 128] = xTg[:, 0:128]
        xk[:, OC_1024] = xTg[:, 1024]
        xk[:, OC_TLEAF:OC_TLEAF + 8] = xTg[:, 8193:8201]
        mask = np.ones((H, SB), NPBF)
        if nreal < NLEAF:
            mask[:, SB - (NLEAF - nreal):] = 0.0
        in_maps.append({"xT": xk, "wcat": wcat, "uiou": uiou, "uf": uf,
                        "bias": bias, "mask": mask})
    return in_maps


def _assemble(results):
    h = np.empty((N, H), np.float32)
    c = np.empty((N, H), np.float32)
    for k in range(NCORE):
        ho = np.asarray(results[k]["h_out"]).astype(np.float32)
        co = np.asarray(results[k]["c_out"]).astype(np.float32)
        lo = 8201 + NLEAF * k
        hi = min(lo + NLEAF, N)
        h[lo:hi] = ho[:, 0:hi - lo].T
        c[lo:hi] = co[:, 0:hi - lo].T
        h[1025 + NL1 * k:1921 + NL1 * k] = ho[:, OC_L1:OC_L1 + NL1].T
        c[1025 + NL1 * k:1921 + NL1 * k] = co[:, OC_L1:OC_L1 + NL1].T
        h[128 + NL2 * k:240 + NL2 * k] = ho[:, OC_L2:OC_L2 + NL2].T
        c[128 + NL2 * k:240 + NL2 * k] = co[:, OC_L2:OC_L2 + NL2].T
    ho = np.asarray(results[0]["h_out"]).astype(np.float32)
    co = np.asarray(results[0]["c_out"]).astype(np.float32)
    h[0:128] = ho[:, OC_TAIL:OC_TAIL + 128].T
    c[0:128] = co[:, OC_TAIL:OC_TAIL + 128].T
    h[1024] = ho[:, OC_1024]
    c[1024] = co[:, OC_1024]
    h[8193:8201] = ho[:, OC_TLEAF:OC_TLEAF + 8].T
    c[8193:8201] = co[:, OC_TLEAF:OC_TLEAF + 8].T
    return h, c


def run(in_maps, **kw):
    nc = _get_program()
    return bass_utils.run_bass_kernel_spmd(nc, in_maps, core_ids=list(range(NCORE)), **kw)


def kernel(x, W_iou, U_iou, b_iou, W_f, U_f, b_f,
           edge_src=None, edge_dst=None, edge_level=None, node_level=None,
           num_levels=None):
    in_maps = _host_prep(x, W_iou, U_iou, b_iou, W_f, U_f, b_f)
    res = run(in_maps)
    return _assemble(res.results)
